# revision 23
# baseline (speedup 1.0000x reference)
"""Trainium2 Bass kernel for nn_DiscoveryMemorywithDynamicThreshold.

Reference computation (batch of 32 samples):
  1. 1x1 conv projection 512->256 channels (+bias)          proj = W @ feats + b
  2. preds-masked average pool over HW                       pooled[b] = mean_l(proj*preds)
  3. sequential memory-bank update over the 32 samples       (cos-sim match -> EMA or append)
  4. cross-attention of proj against the memory bank         aug = mem^T softmax(mem @ proj)
  5. output = concat([proj, aug], channel axis)

Sharding: data-parallel over batch (4 batches per core x 8 cores); the tiny
pooled vectors are AllGathered and the serial scan runs redundantly per core.

v3 design notes:
  - heavy matmuls in bf16 (fp32 streams 2 cycles/col on the PE, bf16 one);
    feats arrive as plain f32 HWDGE DMAs (SWDGE cast-DMA measured ~155 GB/s,
    half rate) and are cast to bf16 on ACT/DVE, which have slack in phase 1.
  - proj is kept in SBUF as bf16; during the collective+scan window it is
    cast back to f32 staging chunks (GpSimd/ACT) and written with plain
    sync DMAs.  aug is copied PSUM->f32 staging directly and sync-written.
  - the scan works in the Gram basis: decisions from
    c = relu(R)*R - thr^2*xsq_i*n2 (no division/sqrt); slot updates are
    uniform col' = (1-a)col + a*d0col; n2 via two predicated copies; the
    coefficient matrix is rebuilt post-scan from the logged a-rows via
    ln -> suffix matmul -> exp (slot overwrites map to ln(0) -> coef 0).
"""

import sys

if "/opt/trn_rl_repo" not in sys.path:
    sys.path.insert(0, "/opt/trn_rl_repo")

import numpy as np

import concourse.bacc as bacc
import concourse.bass as bass
import concourse.tile as tile
from concourse import mybir
from concourse.bass_utils import run_bass_kernel_spmd

F32 = mybir.dt.float32
BF16 = mybir.dt.bfloat16
U8 = mybir.dt.uint8
OP = mybir.AluOpType
ACT = mybir.ActivationFunctionType
X = mybir.AxisListType.X

N_CORES = 8
B_FULL = 32
B_SH = B_FULL // N_CORES          # 4 batches per core
C_IN = 512
C_OUT = 256
HW = 4096
S = 32                            # reachable memory slots (<= batch)
L = 512                           # l-tile
N_LT = HW // L                    # 8 l-tiles per batch
FC = 1024                         # feats DMA chunk columns
BIG = 1.0e30
DECAY = 0.9


def _build(threshold: float):
    nc = bacc.Bacc("TRN2", target_bir_lowering=False, debug=False,
                   num_devices=N_CORES)

    feats_t = nc.dram_tensor("feats", [B_SH, C_IN, HW], F32, kind="ExternalInput")
    preds_t = nc.dram_tensor("preds", [B_SH, HW], F32, kind="ExternalInput")
    w_t = nc.dram_tensor("w", [C_OUT, C_IN], F32, kind="ExternalInput")
    b_t = nc.dram_tensor("b", [C_OUT], F32, kind="ExternalInput")
    ident_t = nc.dram_tensor("ident", [128, 128], F32, kind="ExternalInput")
    shift_t = nc.dram_tensor("shiftI", [S, S], F32, kind="ExternalInput")
    ut_t = nc.dram_tensor("ut", [S, S], F32, kind="ExternalInput")
    cmask_t = nc.dram_tensor("cmask", [S, 32 * N_LT], BF16, kind="ExternalInput")
    bmask_t = nc.dram_tensor("bmask", [S, 32 * N_LT], BF16, kind="ExternalInput")
    out_t = nc.dram_tensor("out", [B_SH, 2 * C_OUT, HW], F32, kind="ExternalOutput")

    thr2 = float(threshold) * float(threshold)

    with tile.TileContext(nc) as tc:
        with (
            tc.tile_pool(name="persist", bufs=1) as persist,
            tc.tile_pool(name="state", bufs=1) as state,
        ):
            # ---------- persistent SBUF ----------
            id_sb = persist.tile([128, 128], F32)
            nc.sync.dma_start(id_sb[:], ident_t[:])
            i32 = id_sb[:32, :32]

            shift_sb = persist.tile([S, S], F32)
            nc.sync.dma_start(shift_sb[:], shift_t[:])
            ut_sb = persist.tile([S, S], F32)
            nc.sync.dma_start(ut_sb[:], ut_t[:])
            cmask_sb = persist.tile([S, 32 * N_LT], BF16)
            nc.sync.dma_start(cmask_sb[:], cmask_t[:])
            bmask_sb = persist.tile([S, 32 * N_LT], BF16)
            nc.sync.dma_start(bmask_sb[:], bmask_t[:])

            ones_bf = persist.tile([1, 128], BF16)
            nc.vector.memset(ones_bf[:], 1.0)
            ones1s = persist.tile([1, S], F32)
            nc.vector.memset(ones1s[:], 1.0)
            ones_col = persist.tile([S, 1], F32)
            nc.vector.memset(ones_col[:], 1.0)
            one1 = persist.tile([1, 1], F32)
            nc.vector.memset(one1[:], 1.0)

            bcol = persist.tile([128, 2], F32)
            for oh in range(2):
                nc.sync.dma_start(
                    bcol[:, oh:oh + 1],
                    b_t[oh * 128:(oh + 1) * 128].rearrange("(p o) -> p o", o=1),
                )

            # W^T (conv lhsT) in bf16, via PE transpose of f32 W chunks
            wt_bf = persist.tile([128, 4 * C_OUT], BF16)
            proj_sb0 = persist.tile([128, B_SH * HW], BF16)
            proj_sb1 = persist.tile([128, B_SH * HW], BF16)
            proj_sb = [proj_sb0, proj_sb1]

            # pooled / collective staging
            pooled_sb = state.tile([128, 2 * B_SH], F32)     # [c-half, 2*b]
            pooledT_sb = state.tile([B_SH, C_OUT], F32)
            pag_sb = state.tile([B_FULL, C_OUT], F32)
            pag_bf = state.tile([B_FULL, C_OUT], BF16)
            pcb_sb = state.tile([128, 2 * B_FULL], F32)
            pcb_bf = state.tile([128, 2 * B_FULL], BF16)

            # scan constants/state
            d0_sb = state.tile([S, S], F32)
            dcol = state.tile([S, 1], F32)
            xsq_sb = state.tile([1, S], F32)
            th2_sb = state.tile([1, S], F32)
            xq2_sb = state.tile([1, S], F32)
            sd_sb = state.tile([1, S], F32)
            m_sb = state.tile([S, S], F32)                   # M[j,s]=<x_j,mem_s>
            n2 = state.tile([1, S], F32)
            p1h = state.tile([1, S], F32)
            sh = state.tile([1, S], F32)
            amat_f = state.tile([1, S * S], F32)             # a-rows, flat
            amat = state.tile([S, S], F32)
            coefT_sb = state.tile([S, S], F32)
            coefT_bf = state.tile([S, S], BF16)
            mem_bf = state.tile([S, C_OUT], BF16)
            penc_sb = state.tile([S, 1], F32)
            g_sb = persist.tile([S, B_SH * HW], BF16)
            rcpd = state.tile([S, B_SH * L], BF16)
            nc.vector.memset(rcpd[:], 0.0)
            e_sb = persist.tile([S, B_SH * HW], BF16)

            with tc.tile_pool(name="prep_ps", bufs=2, space="PSUM") as prep_ps:
                with tc.tile_pool(name="wtmp", bufs=1) as wtmp:
                    w_sb = wtmp.tile([128, 2 * C_IN], F32)
                    for oh in range(2):
                        nc.sync.dma_start(
                            w_sb[:, oh * C_IN:(oh + 1) * C_IN],
                            w_t[oh * 128:(oh + 1) * 128, :],
                        )
                    for oh in range(2):
                        for kc in range(4):
                            tp = prep_ps.tile([128, 128], F32, tag="prep")
                            nc.tensor.transpose(
                                tp[:],
                                w_sb[:, oh * C_IN + kc * 128: oh * C_IN + (kc + 1) * 128],
                                id_sb[:],
                            )
                            nc.vector.tensor_copy(
                                wt_bf[:, kc * C_OUT + oh * 128: kc * C_OUT + (oh + 1) * 128],
                                tp[:],
                            )

                # ---------- phase 1: conv + masked pooling ----------
                with (
                    tc.tile_pool(name="fpool", bufs=2) as fpool,
                    tc.tile_pool(name="fbpool", bufs=2) as fbpool,
                    tc.tile_pool(name="prpool", bufs=1) as prpool,
                    tc.tile_pool(name="scrpool", bufs=2) as scrpool,
                    tc.tile_pool(name="pcpool", bufs=2) as pcpool,
                    tc.tile_pool(name="conv_ps", bufs=3, space="PSUM") as conv_ps,
                    tc.tile_pool(name="pbc_ps", bufs=2, space="PSUM") as pbc_ps,
                ):
                    for b in range(B_SH):
                        prow = prpool.tile([1, HW], BF16, tag="prow")
                        nc.gpsimd.dma_start(prow[:], preds_t[b:b + 1, :])
                        pc0 = pcpool.tile([128, N_LT], F32, tag="pc0")
                        pc1 = pcpool.tile([128, N_LT], F32, tag="pc1")
                        pcs = [pc0, pc1]
                        for h in range(4):          # 4 chunks of 1024 cols
                            fch = []
                            for kc in range(4):
                                f = fpool.tile([128, FC], F32, tag=f"f{kc}")
                                nc.sync.dma_start(
                                    f[:],
                                    feats_t[b, kc * 128:(kc + 1) * 128,
                                            h * FC:(h + 1) * FC],
                                )
                                fb = fbpool.tile([128, FC], BF16, tag=f"fb{kc}")
                                if kc % 2 == 0:
                                    nc.vector.tensor_copy(fb[:], f[:])
                                else:
                                    nc.scalar.copy(fb[:], f[:])
                                fch.append(fb)
                            for lt2 in range(2):
                                lt = h * 2 + lt2
                                col = b * N_LT + lt
                                pbc = pbc_ps.tile([128, L], F32, tag="pbc")
                                nc.tensor.matmul(
                                    pbc[:], ones_bf[:, :128],
                                    prow[:, lt * L:(lt + 1) * L],
                                    start=True, stop=True,
                                )
                                for oh in range(2):
                                    ps = conv_ps.tile([128, L], F32, tag="cv")
                                    for kc in range(4):
                                        nc.tensor.matmul(
                                            ps[:],
                                            wt_bf[:, kc * C_OUT + oh * 128:
                                                     kc * C_OUT + (oh + 1) * 128],
                                            fch[kc][:, lt2 * L:(lt2 + 1) * L],
                                            start=(kc == 0), stop=(kc == 3),
                                        )
                                    pslice = proj_sb[oh][:, col * L:(col + 1) * L]
                                    nc.scalar.activation(
                                        pslice, ps[:], ACT.Identity,
                                        bias=bcol[:, oh:oh + 1], scale=1.0,
                                    )
                                    scr = scrpool.tile([128, L], F32, tag="scr")
                                    nc.vector.scalar_tensor_tensor(
                                        scr[:], pslice, 1.0 / HW, pbc[:],
                                        OP.mult, OP.mult,
                                        accum_out=pcs[oh][:, lt:lt + 1],
                                    )
                        for oh in range(2):
                            nc.vector.reduce_sum(
                                pooled_sb[:, oh * B_SH + b: oh * B_SH + b + 1],
                                pcs[oh][:], X,
                            )

                # ---------- phase 1b: allgather pooled ----------
                for oh in range(2):
                    tp = prep_ps.tile([B_SH, 128], F32, tag="prep")
                    nc.tensor.transpose(
                        tp[:], pooled_sb[:, oh * B_SH:(oh + 1) * B_SH], id_sb[:]
                    )
                    nc.vector.tensor_copy(
                        pooledT_sb[:, oh * 128:(oh + 1) * 128], tp[:]
                    )

                with (
                    tc.tile_pool(name="dram", bufs=1, space="DRAM") as dram,
                    tc.tile_pool(name="pstage", bufs=4) as pstage,
                ):
                    agin = dram.tile([B_SH, C_OUT], F32)
                    agout = dram.tile([B_FULL, C_OUT], F32)
                    nc.gpsimd.dma_start(agin[:], pooledT_sb[:])
                    nc.gpsimd.collective_compute(
                        "AllGather", OP.bypass,
                        replica_groups=[list(range(N_CORES))],
                        ins=[agin.opt()], outs=[agout.opt()],
                    )

                    # proj write-out fills the collective + scan window:
                    # bf16 -> f32 staging casts on GpSimd/ACT, plain sync DMAs.
                    def proj_out(k):
                        b, oh, hh = k // 4, (k // 2) % 2, k % 2
                        st = pstage.tile([128, HW // 2], F32, tag="pst")
                        src = proj_sb[oh][:, b * HW + hh * (HW // 2):
                                          b * HW + (hh + 1) * (HW // 2)]
                        nc.scalar.copy(st[:], src)
                        nc.sync.dma_start(
                            out_t[b, oh * 128:(oh + 1) * 128,
                                  hh * (HW // 2):(hh + 1) * (HW // 2)],
                            st[:],
                        )

                    for k in range(6):
                        proj_out(k)
                    nc.gpsimd.dma_start(pag_sb[:], agout[:])
                    for k in range(6, 10):
                        proj_out(k)

                    nc.vector.tensor_copy(pag_bf[:], pag_sb[:])

                    for oh in range(2):
                        tp2 = prep_ps.tile([128, B_FULL], F32, tag="prep")
                        nc.tensor.transpose(
                            tp2[:], pag_sb[:, oh * 128:(oh + 1) * 128], i32
                        )
                        nc.vector.tensor_copy(
                            pcb_sb[:, oh * B_FULL:(oh + 1) * B_FULL], tp2[:]
                        )
                        nc.vector.tensor_copy(
                            pcb_bf[:, oh * B_FULL:(oh + 1) * B_FULL], tp2[:]
                        )

                    d0ps = prep_ps.tile([S, S], F32, tag="prep")
                    for oh in range(2):
                        pc = pcb_sb[:, oh * B_FULL:(oh + 1) * B_FULL]
                        nc.tensor.matmul(d0ps[:], pc, pc,
                                         start=(oh == 0), stop=(oh == 1))
                    nc.vector.tensor_copy(d0_sb[:], d0ps[:])

                    scr32 = state.tile([S, S], F32)
                    nc.vector.scalar_tensor_tensor(
                        scr32[:], d0_sb[:], 1.0, i32, OP.mult, OP.mult,
                        accum_out=dcol[:],
                    )
                    xsqps = prep_ps.tile([1, S], F32, tag="prep")
                    nc.tensor.matmul(xsqps[:], dcol[:], i32, start=True, stop=True)
                    nc.vector.tensor_copy(xsq_sb[:], xsqps[:])
                    nc.vector.tensor_scalar(th2_sb[:], xsq_sb[:], thr2, None, OP.mult)
                    nc.vector.tensor_scalar(xq2_sb[:], xsq_sb[:],
                                            (1.0 - DECAY) * (1.0 - DECAY),
                                            None, OP.mult)
                    # sub-diagonal sd[i] = d0[i+1, i]
                    nc.vector.tensor_mul(scr32[:], d0_sb[:], shift_sb[:])
                    sdps = prep_ps.tile([1, S], F32, tag="prep")
                    nc.tensor.matmul(sdps[:], ones_col[:], scr32[:],
                                     start=True, stop=True)
                    nc.vector.tensor_copy(sd_sb[:], sdps[:])

                    # scan init (step 0 always appends into slot 0)
                    nc.vector.memset(m_sb[:], 0.0)
                    nc.vector.tensor_copy(m_sb[:, 0:1], d0_sb[:, 0:1])
                    nc.vector.memset(n2[:], BIG)
                    nc.vector.tensor_copy(n2[:, 0:1], xsq_sb[:, 0:1])
                    nc.vector.memset(p1h[:], 0.0)
                    nc.vector.memset(p1h[:, 1:2], 1.0)
                    nc.vector.memset(sh[:], 0.0)
                    nc.vector.memset(amat_f[:], 0.0)
                    nc.vector.memset(amat_f[:, 0:1], 1.0)

                    # ---------- phase 2: serial scan over samples 1..31 ----------
                    with (
                        tc.tile_pool(name="rows", bufs=3) as rows,
                        tc.tile_pool(name="rx_ps", bufs=2, space="PSUM") as rx_ps,
                        tc.tile_pool(name="bca_ps", bufs=2, space="PSUM") as bca_ps2,
                        tc.tile_pool(name="ka_ps", bufs=1, space="PSUM") as ka_ps,
                    ):
                        # PE keep-alive: the HAM throttles the PE to 1.2 GHz
                        # after ~3.4us idle and (observed) can stay stuck cold
                        # through the whole attention phase.  Dummy matmuls
                        # sandwiched between the scan's real PE ops keep the
                        # activity monitor busy so phase 3 runs at 2.4 GHz.
                        def ka(n=3):
                            kt = ka_ps.tile([128, L], F32, tag="ka")
                            for _ in range(n):
                                nc.tensor.matmul(kt[:], wt_bf[:, 0:128],
                                                 proj_sb0[:, 0:L],
                                                 start=True, stop=True)

                        def g_tile(ct):
                            gp = ka_ps.tile([S, L], F32, tag="g")
                            for oh in range(2):
                                nc.tensor.matmul(
                                    gp[:],
                                    pcb_bf[:, oh * B_FULL:(oh + 1) * B_FULL],
                                    proj_sb[oh][:, ct * L:(ct + 1) * L],
                                    start=(oh == 0), stop=(oh == 1),
                                )
                            nc.scalar.copy(g_sb[:, ct * L:(ct + 1) * L], gp[:])

                        r_prev = rows.tile([1, S], F32, tag="R")
                        nc.vector.memset(r_prev[:], 0.0)
                        nc.vector.tensor_copy(r_prev[0:1, 0:1], sd_sb[0:1, 0:1])

                        for k in range(10, 16):
                            proj_out(k)
                        ka(12)

                        for i in range(1, B_FULL):
                            a_sl = amat_f[:, i * S:(i + 1) * S]
                            # off-critical feeders
                            rxp = rx_ps.tile([1, S], F32, tag="rx")
                            if i < B_FULL - 1:
                                nc.tensor.matmul(rxp[:], id_sb[:32, i + 1:i + 2],
                                                 m_sb[:], start=True, stop=True)
                            z0 = rows.tile([1, S], F32, tag="z0")
                            nc.scalar.activation(z0[:], n2[:], ACT.Copy,
                                                 scale=DECAY * DECAY)
                            z1 = rows.tile([1, S], F32, tag="z1")
                            nc.scalar.activation(z1[:], r_prev[:], ACT.Identity,
                                                 scale=2.0 * DECAY * (1.0 - DECAY),
                                                 bias=xq2_sb[0:1, i:i + 1])
                            xb = rows.tile([1, S], F32, tag="xb")
                            nc.scalar.activation(xb[:], ones1s[:], ACT.Copy,
                                                 scale=xsq_sb[0:1, i:i + 1])
                            nc.scalar.copy(sh[0:1, 1:S], p1h[0:1, 0:S - 1])
                            znew = rows.tile([1, S], F32, tag="znew")
                            nc.gpsimd.tensor_add(znew[:], z0[:], z1[:])

                            # critical DVE chain (negated compare domain:
                            # negc = th2*n2 - relu(R)*R, decisions via min)
                            q = rows.tile([1, S], F32, tag="q")
                            nc.vector.scalar_tensor_tensor(q[:], r_prev[:], 0.0,
                                                           r_prev[:], OP.max, OP.mult)
                            negc = rows.tile([1, S], F32, tag="negc")
                            nc.vector.scalar_tensor_tensor(
                                negc[:], n2[:], th2_sb[0:1, i:i + 1], q[:],
                                OP.mult, OP.subtract)
                            mxc = rows.tile([1, 1], F32, tag="mxc")
                            nc.vector.tensor_reduce(mxc[:], negc[:], X, OP.min)
                            mxp = rows.tile([1, 1], F32, tag="mxp")
                            nc.vector.tensor_scalar(mxp[:], mxc[:], 0.0, None, OP.min)
                            cnt = rows.tile([1, 1], F32, tag="cnt")
                            mske = rows.tile([1, S], U8, tag="mske")
                            nc.vector.tensor_scalar(mske[:], negc[:], mxp[0:1, 0:1],
                                                    1.0, OP.is_le, OP.mult,
                                                    accum_out=cnt[:])
                            mska = rows.tile([1, S], U8, tag="mska")
                            nc.vector.tensor_scalar(mska[:], p1h[:], cnt[0:1, 0:1],
                                                    0.5, OP.subtract, OP.is_ge)
                            nd = rows.tile([1, 1], F32, tag="nd")
                            nc.vector.tensor_scalar(nd[:], cnt[:], 0.5, None,
                                                    OP.is_le)

                            t4p = rows.tile([1, S], F32, tag="t4p")
                            nc.vector.tensor_scalar(t4p[:], p1h[:], nd[0:1, 0:1],
                                                    None, OP.mult)
                            # a-row: (1-D)*mske + (1-d)*p1h, straight into amat
                            nc.vector.scalar_tensor_tensor(
                                a_sl, mske[:], 1.0 - DECAY, t4p[:],
                                OP.mult, OP.add)

                            if i < B_FULL - 1:
                                rx = rows.tile([1, S], F32, tag="rxs")
                                nc.scalar.copy(rx[:], rxp[:])
                                t2 = rows.tile([1, S], F32, tag="t2")
                                nc.vector.scalar_tensor_tensor(
                                    t2[:], rx[:], sd_sb[0:1, i:i + 1], a_sl,
                                    OP.subtract, OP.mult)
                                r_new = rows.tile([1, S], F32, tag="R")
                                nc.vector.tensor_sub(r_new[:], rx[:], t2[:])
                                r_prev = r_new

                            # state updates
                            nc.vector.copy_predicated(n2[:], mska[:], xb[:])
                            nc.vector.copy_predicated(n2[:], mske[:], znew[:])
                            u_sh = rows.tile([1, S], F32, tag="u_sh")
                            nc.gpsimd.tensor_sub(u_sh[:], sh[:], p1h[:])
                            nc.vector.scalar_tensor_tensor(
                                p1h[:], u_sh[:], nd[0:1, 0:1], p1h[:],
                                OP.mult, OP.add)
                            if i < B_FULL - 1:
                                bca = bca_ps2.tile([S, S], F32, tag="bca")
                                nc.tensor.matmul(bca[:], ones1s[:], a_sl,
                                                 start=True, stop=True)
                                dm = rows.tile([S, S], F32, tag="dm")
                                nc.vector.scalar_tensor_tensor(
                                    dm[:], m_sb[:], d0_sb[:, i:i + 1], bca[:],
                                    OP.subtract, OP.mult)
                                nc.gpsimd.tensor_sub(m_sb[:], m_sb[:], dm[:])
                            if i - 1 < B_SH * N_LT:
                                g_tile(i - 1)

                        for ct in range(B_FULL - 1, B_SH * N_LT):
                            g_tile(ct)

            # ---------- phase 2b: coef reconstruction + memory build ----------
            with (
                tc.tile_pool(name="post_ps", bufs=2, space="PSUM") as post_ps,
                tc.tile_pool(name="post_dram", bufs=1, space="DRAM") as post_dram,
            ):
                # reshape flat a-rows to [iter, slot] via a DRAM bounce
                amat_d = post_dram.tile([1, S * S], F32)
                nc.sync.dma_start(amat_d[:], amat_f[:])
                nc.sync.dma_start(
                    amat[:], amat_d[:].rearrange("o (i s) -> (o i) s", s=S))
                ln1 = state.tile([S, S], F32)
                nc.scalar.activation(ln1[:], amat[:], ACT.Ln, bias=1.0, scale=-1.0)
                # clamp -inf (overwritten slots, a=1) so 0-weighted terms of
                # the suffix matmul don't produce 0*inf = NaN
                nc.vector.tensor_scalar(ln1[:], ln1[:], -1.0e4, None, OP.max)
                sfx = post_ps.tile([S, S], F32, tag="post")
                nc.tensor.matmul(sfx[:], ut_sb[:], ln1[:], start=True, stop=True)
                pexp = state.tile([S, S], F32)
                nc.scalar.activation(pexp[:], sfx[:], ACT.Exp)
                nc.vector.tensor_mul(coefT_sb[:], amat[:], pexp[:])
                nc.vector.tensor_copy(coefT_bf[:], coefT_sb[:])
                memp = post_ps.tile([S, C_OUT], F32, tag="post")
                nc.tensor.matmul(memp[:], coefT_bf[:], pag_bf[:],
                                 start=True, stop=True)
                nc.vector.tensor_copy(mem_bf[:], memp[:])
                # slot-validity penalty column for the softmax
                val = state.tile([1, S], F32)
                nc.vector.tensor_scalar(val[:], n2[:], 0.1 * BIG, None, OP.is_lt)
                pen = state.tile([1, S], F32)
                nc.vector.tensor_scalar(pen[:], val[:], 1.0, BIG,
                                        OP.subtract, OP.mult)
                pps = post_ps.tile([S, 1], F32, tag="post")
                nc.tensor.matmul(pps[:], pen[:], one1[:], start=True, stop=True)
                nc.vector.tensor_copy(penc_sb[:], pps[:])

            # ---------- phase 3: cross-attention ----------
            # dense sub-phases keep the PE busy so HAM stays un-throttled:
            # (A) all logits+exp, (B) all denominators, (C) rbc+aug+copies.
            with (
                tc.tile_pool(name="att_sb", bufs=2) as att_sb,
                tc.tile_pool(name="apool", bufs=2) as apool,
                tc.tile_pool(name="lg_ps", bufs=2, space="PSUM") as lg_ps,
                tc.tile_pool(name="den_ps", bufs=2, space="PSUM") as den_ps,
                tc.tile_pool(name="rbc_ps", bufs=2, space="PSUM") as rbc_ps,
                tc.tile_pool(name="aug_ps", bufs=2, space="PSUM") as aug_ps,
            ):
                for col in range(B_SH * N_LT):
                    lg = lg_ps.tile([S, L], F32, tag="lg")
                    nc.tensor.matmul(lg[:], coefT_bf[:],
                                     g_sb[:, col * L:(col + 1) * L],
                                     start=True, stop=True)
                    nc.scalar.activation(e_sb[:, col * L:(col + 1) * L], lg[:],
                                         ACT.Exp, bias=penc_sb[:, 0:1],
                                         scale=1.0)
                for b in range(B_SH):
                    den = den_ps.tile([S, L], F32, tag="den")
                    for lt in range(N_LT):
                        col = b * N_LT + lt
                        nc.tensor.matmul(
                            den[:],
                            cmask_sb[:, lt * 32:(lt + 1) * 32],
                            e_sb[:, col * L:(col + 1) * L],
                            start=(lt == 0), stop=(lt == N_LT - 1),
                        )
                    den_sb = att_sb.tile([N_LT, L], F32, tag="den_sb")
                    nc.vector.tensor_copy(den_sb[:], den[:N_LT, :])
                    rcf = att_sb.tile([N_LT, L], F32, tag="rcf")
                    rcs = att_sb.tile([N_LT, L], F32, tag="rcs")
                    nc.vector.reciprocal_approx_accurate(rcf[:], den_sb[:], rcs[:])
                    nc.vector.tensor_copy(rcpd[:N_LT, b * L:(b + 1) * L], rcf[:])
                HH = HW // 2
                for b in range(B_SH):
                    for hh in range(2):
                        ast0 = apool.tile([128, HH], F32, tag="augst0")
                        ast1 = apool.tile([128, HH], F32, tag="augst1")
                        ast = [ast0, ast1]
                        for lt2 in range(N_LT // 2):
                            lt = hh * (N_LT // 2) + lt2
                            col = b * N_LT + lt
                            rbc = rbc_ps.tile([S, L], F32, tag="rbc")
                            nc.tensor.matmul(rbc[:],
                                             bmask_sb[:, lt * 32:(lt + 1) * 32],
                                             rcpd[:, b * L:(b + 1) * L],
                                             start=True, stop=True)
                            esl = e_sb[:, col * L:(col + 1) * L]
                            nc.vector.tensor_mul(esl, esl, rbc[:])
                            for oh in range(2):
                                aug = aug_ps.tile([128, L], F32, tag="aug")
                                nc.tensor.matmul(
                                    aug[:],
                                    mem_bf[:, oh * 128:(oh + 1) * 128],
                                    esl, start=True, stop=True,
                                )
                                dst = ast[oh][:, lt2 * L:(lt2 + 1) * L]
                                if (2 * lt + oh) % 2 == 0:
                                    nc.scalar.copy(dst, aug[:])
                                else:
                                    nc.vector.tensor_copy(dst, aug[:])
                        for oh in range(2):
                            nc.sync.dma_start(
                                out_t[b, C_OUT + oh * 128:C_OUT + (oh + 1) * 128,
                                      hh * HH:(hh + 1) * HH],
                                ast[oh][:],
                            )

    nc.compile()
    return nc


_CACHE: dict = {}


def _get_program(threshold: float):
    key = round(float(threshold), 9)
    if key not in _CACHE:
        _CACHE[key] = _build(threshold)
    return _CACHE[key]


def _make_consts():
    ident = np.eye(128, dtype=np.float32)
    shiftI = np.zeros((S, S), dtype=np.float32)
    for i in range(S - 1):
        shiftI[i + 1, i] = 1.0
    ut = np.zeros((S, S), dtype=np.float32)
    for bb in range(S):
        ut[bb + 1:, bb] = 1.0
    cmask = np.zeros((S, 32 * N_LT), dtype=np.float32)
    bmask = np.zeros((S, 32 * N_LT), dtype=np.float32)
    for t in range(N_LT):
        cmask[:, 32 * t + t] = 1.0
        bmask[t, 32 * t:32 * (t + 1)] = 1.0
    return ident, shiftI, ut, cmask, bmask


def _bf16(x):
    import ml_dtypes
    return x.astype(ml_dtypes.bfloat16)


def _make_inmaps(feats, preds, W, b):
    ident, shiftI, ut, cmask, bmask = _make_consts()
    feats_r = feats.reshape(B_FULL, C_IN, HW)
    preds_r = preds.reshape(B_FULL, HW)
    in_maps = []
    for r in range(N_CORES):
        lo, hi = r * B_SH, (r + 1) * B_SH
        in_maps.append({
            "feats": feats_r[lo:hi],
            "preds": preds_r[lo:hi],
            "w": W,
            "b": b,
            "ident": ident,
            "shiftI": shiftI,
            "ut": ut,
            "cmask": _bf16(cmask),
            "bmask": _bf16(bmask),
        })
    return in_maps


def kernel(feats, preds, W, b, epoch):
    feats = np.ascontiguousarray(np.asarray(feats, dtype=np.float32))
    preds = np.ascontiguousarray(np.asarray(preds, dtype=np.float32))
    W = np.ascontiguousarray(np.asarray(W, dtype=np.float32))
    b = np.ascontiguousarray(np.asarray(b, dtype=np.float32))
    epoch = int(np.asarray(epoch))

    threshold = (epoch / 10 - 2) * 0.4 / 13 + 0.3
    assert threshold > 0.0, "kernel assumes a positive match threshold"

    B, C, H, Wd = feats.shape
    assert (B, C, H * Wd) == (B_FULL, C_IN, HW)

    nc = _get_program(threshold)
    in_maps = _make_inmaps(feats, preds, W, b)
    res = run_bass_kernel_spmd(nc, in_maps, core_ids=list(range(N_CORES)))
    out = np.concatenate([res.results[r]["out"] for r in range(N_CORES)], axis=0)
    return out.reshape(B_FULL, 2 * C_OUT, H, Wd)
